# revision 88
# baseline (speedup 1.0000x reference)
"""Causal GQA self-attention block (B=4, T=2048, C=1024, H=16, G=4) on 8
Trainium2 NeuronCores.

Sharding: core c = d*4+g  (d in {0,1} batch-DP, g in {0..3} kv-group TP).
Each core handles batches [2d, 2d+1], heads {g, g+4, g+8, g+12}, kv group g,
and produces a partial projection output; the host sums the 4 TP partials
per batch pair and adds the bias.

Per-core kernel (bf16 operands, fp32 PSUM accumulation):
  - fused QKV projection from pre-transposed x (host supplies x^T),
    producing Q^T / K^T / V^T with channels on partitions
  - x loaded in (tq-tile, cc) order on the vector DMA ring so the first
    QKV matmul can start ~6us in instead of after the full 8MB load
  - scores computed transposed (S^T[tk,tq] = K Q^T) in 128x512 tiles,
    head-pair packed into the PE array via tile_position (contraction=64)
  - causal: block skip + column trim + additive -1e30 band mask applied
    by an extra accumulating matmul (maskA^T @ I) into the scores PSUM
  - unnormalized softmax: exp on ACT (scale folded), denominator obtained
    by a leading ones-column in the P@V matmul (M=65, den on partition 0)
  - normalize via DVE reciprocal + gpsimd partition-broadcast + DVE mult
  - output projection on-device (fp16 partials); host sums TP partials
  - cross-batch software pipeline: batch b's projection chunks execute
    inside batch b+1's attention slots so the ACT (exp) stream never gaps
"""

import os
import sys

sys.path.insert(0, "/opt/trn_rl_repo")

import numpy as np
from contextlib import ExitStack

import concourse.bass as bass
import concourse.mybir as mybir
import concourse.tile as tile
from concourse import bacc
from concourse.bass_utils import run_bass_kernel_spmd

# problem shape (hardcoded per contract)
B, T, C = 4, 2048, 1024
H, G = 16, 4
D = C // H  # 64

# per-core
B_LOC = 2        # batches per core
NPAIR = 2        # head pairs per core (4 heads)
P = 128
CC = C // P      # 8 contraction chunks for projections
NT = 512         # tq tile width
TQT = T // NT    # 4 tq tiles
TKC = T // P     # 16 tk chunks
NEG = -1.0e30

F32 = mybir.dt.float32
F16 = mybir.dt.float16
BF16 = mybir.dt.bfloat16
F8 = mybir.dt.float8e4
ADT = BF16
Exp = mybir.ActivationFunctionType.Exp
ADD = mybir.AluOpType.add
MULT = mybir.AluOpType.mult


def _build_program():
    nc = bacc.Bacc(None, target_bir_lowering=False)

    # x^T with the partition dim outermost: xT[b, p, cc, t] = x[b, t, cc*128+p]
    xT = nc.dram_tensor("xT", [B_LOC, P, CC, T], ADT, kind="ExternalInput")
    # columns: q pair0 (128) | q pair1 (128) | k (64) | v (64);
    # contraction rows pre-split [P, CC] so each loads as a single DMA
    wqkv = nc.dram_tensor("wqkv", [P, CC, 384], ADT, kind="ExternalInput")
    wproj = nc.dram_tensor("wproj", [P, 2, C], ADT, kind="ExternalInput")
    # additive causal mask: maska[q, k] = -1e30 where k > q else 0
    maska = nc.dram_tensor("maska", [P, P], ADT, kind="ExternalInput")
    i128 = nc.dram_tensor("i128", [P, P], ADT, kind="ExternalInput")
    ident2 = nc.dram_tensor("ident2", [P, 64], ADT, kind="ExternalInput")
    vones = nc.dram_tensor("vones", [P, TKC], ADT, kind="ExternalInput")
    vones8 = nc.dram_tensor("vones8", [P, TKC // 2, 2], F8, kind="ExternalInput")
    outp = nc.dram_tensor("outp", [B_LOC, T, C], F16, kind="ExternalOutput")

    with tile.TileContext(nc) as tc:
        with ExitStack() as ctx:
            const = ctx.enter_context(tc.tile_pool(name="const", bufs=1))
            sb = ctx.enter_context(tc.tile_pool(name="sb", bufs=1))
            sb2 = ctx.enter_context(tc.tile_pool(name="sb2", bufs=2))
            xp = ctx.enter_context(tc.tile_pool(name="xp", bufs=2))
            small = ctx.enter_context(tc.tile_pool(name="small", bufs=4))
            ppool = ctx.enter_context(tc.tile_pool(name="ppool", bufs=6))
            stg = ctx.enter_context(tc.tile_pool(name="stg", bufs=4))
            ps_st = ctx.enter_context(tc.tile_pool(name="ps_st", bufs=2, space="PSUM"))
            ps_pv = ctx.enter_context(tc.tile_pool(name="ps_pv", bufs=2, space="PSUM"))
            ps_mm = ctx.enter_context(tc.tile_pool(name="ps_mm", bufs=2, space="PSUM"))

            # ---- constants (sync ring; x tiles go on the vector ring) ----
            wqkv_t = const.tile([P, CC, 384], ADT, tag="wqkv")
            nc.sync.dma_start(wqkv_t[:], wqkv[:])
            maska_t = const.tile([P, P], ADT, tag="maska")
            nc.sync.dma_start(maska_t[:], maska[:])
            i128_t = const.tile([P, P], ADT, tag="i128")
            nc.sync.dma_start(i128_t[:], i128[:])
            id2_t = const.tile([P, 64], ADT, tag="ident2")
            nc.sync.dma_start(id2_t[:], ident2[:])
            # wproj is first needed by the projections mid-batch-0: keep it
            # off the startup critical path (scalar ring, behind the x tiles)
            wproj_t = const.tile([P, 2, C], ADT, tag="wproj")

            def emit_wproj_load():
                nc.sync.dma_start(wproj_t[:], wproj[:])

            def emit_setup(b, first=False):
                # ---- load x^T for this batch + allocate state ----
                # All x loads share the sync ring: FIFO order is the QoS —
                # batch 0's tile-0 chunks lead the ring (fine-grained so the
                # first QKV matmuls start asap); later/batch-1 tiles are big
                # single DMAs that fill ring idle behind dependent traffic.
                xt = xp.tile([P, CC, T], ADT, tag="xt")
                if first:
                    for cc in range(CC):
                        nc.sync.dma_start(xt[:, cc, 0:NT], xT[b, :, cc, 0:NT])
                else:
                    nc.sync.dma_start(xt[:, :, 0:NT], xT[b, :, :, 0:NT])
                for n in range(1, TQT):
                    nc.sync.dma_start(
                        xt[:, :, n * NT : (n + 1) * NT],
                        xT[b, :, :, n * NT : (n + 1) * NT],
                    )
                # q_sb[:, p, t]: pair p -> heads (2p, 2p+1) at rows 0:64 / 64:128
                q_sb = sb2.tile([P, NPAIR, T], ADT, tag="q")
                # kv_sb rows 0:64 = K^T (kv-group), rows 64:128 = V^T
                kv_sb = sb2.tile([P, TQT, NT], ADT, tag="kv")
                k_hi = sb2.tile([P, TQT, NT], ADT, tag="khi")  # K dup at rows 64:128
                # ones columns via gpsimd memset — keeps them off the sync
                # DMA ring and its 8 shared semaphore lanes at startup
                v_a = sb2.tile([P, TKC, 65], ADT, tag="va")
                nc.gpsimd.memset(v_a[:, :, 64], 1.0)
                # fp8 copy of V, tk-chunk pairs interleaved for DoubleRow
                # (col stride 80 keeps the pair-dim step 16B-aligned)
                v8 = sb2.tile([P, TKC // 2, 2, 80], F8, tag="v8")
                nc.gpsimd.memset(v8[:, :, :, 64], 1.0)
                o_t = sb2.tile([P, NPAIR, T], ADT, tag="ot", name=f"ot{b}")
                return xt, q_sb, kv_sb, k_hi, v_a, o_t, v8

            def emit_qkv_part(b, st8, n, part):
                # ---- QKV projection tile n, sub-part (0: kv proj + V
                # transpose + k dup, 1: q pair0 proj, 2: q pair1 proj) ----
                xt, q_sb, kv_sb, k_hi, v_a, o_t, v8 = st8
                m = {0: 2, 1: 0, 2: 1}[part]
                pm = ps_mm.tile([P, NT], F32, tag="mm")
                for cc in range(CC):
                    nc.tensor.matmul(
                        pm[:],
                        wqkv_t[:, cc, m * P : (m + 1) * P],
                        xt[:, cc, n * NT : (n + 1) * NT],
                        start=(cc == 0),
                        stop=(cc == CC - 1),
                    )
                if m < 2:
                    nc.vector.tensor_copy(q_sb[:, m, n * NT : (n + 1) * NT], pm[:])
                    return
                nc.vector.tensor_copy(kv_sb[:, n, :], pm[:])
                # batch 0 tile 0's K-dup rides the scalar ring: ACT is idle
                # until the first exp, which itself depends on this dup
                keng = nc.scalar if (b == 0 and n == 0) else nc.sync
                keng.dma_start(k_hi[64:128, n, :], kv_sb[0:64, n, :])
                for i in range(4 * n, 4 * n + 4):
                    pt = ps_mm.tile([P, 64], ADT, tag="mm")
                    nc.tensor.transpose(
                        pt[:],
                        kv_sb[64:128, i // 4, (i % 4) * P : (i % 4 + 1) * P],
                        id2_t[64:128, :],
                    )
                    nc.vector.tensor_copy(v_a[:, i, 0:64], pt[:])

            def emit_attn_jp(b, st8, j, p_, tail=False):
                xt, q_sb, kv_sb, k_hi, v_a, o_t, v8 = st8
                pv = [
                    ps_pv.tile([P, NT], F32, tag="pv", name=f"pv{e}")
                    for e in range(2)
                ]
                last = 4 * j + 3
                p8 = None
                for i in range(4 * j + 4):
                    diag = i >= 4 * j
                    r = i - 4 * j
                    lo = r * P if diag else 0
                    st = ps_st.tile([P, 2, NT], F32, tag="st")
                    for e in range(2):
                        ksrc = kv_sb if e == 0 else k_hi
                        nc.tensor.matmul(
                            st[:, e, lo:NT],
                            ksrc[
                                64 * e : 64 * e + 64,
                                i // 4,
                                (i % 4) * P : (i % 4 + 1) * P,
                            ],
                            q_sb[
                                64 * e : 64 * e + 64,
                                p_,
                                j * NT + lo : (j + 1) * NT,
                            ],
                            start=True,
                            stop=not diag,
                            tile_position=(64 * e, 0),
                        )
                    if diag:
                        # additive causal band mask: st[:,e,lo:lo+P] += A^T I
                        for e in range(2):
                            nc.tensor.matmul(
                                st[:, e, lo : lo + P],
                                maska_t[:],
                                i128_t[:],
                                start=False,
                                stop=True,
                            )
                        pexp = ppool.tile([P, 2, NT], ADT, tag="pexp")
                        nc.scalar.activation(
                            pexp[:, :, lo:NT],
                            st[:, :, lo:NT],
                            Exp,
                            scale=0.125,
                        )
                        for e in range(2):
                            nc.tensor.matmul(
                                pv[e][0:65, lo:NT],
                                v_a[:, i, :],
                                pexp[:, e, lo:NT],
                                start=(i == 0),
                                stop=(i == last),
                            )
                    else:
                        # off-diagonal tk chunks: exp in fp8, P@V fused two
                        # chunks at a time via DoubleRow (2 fp8 MACs/cell)
                        if i % 2 == 0:
                            p8 = ppool.tile([P, 2, 2, NT], F8, tag="pexp8")
                        nc.scalar.activation(
                            p8[:, i % 2, :, :],
                            st[:, :, :],
                            Exp,
                            scale=0.125,
                        )
                        if i % 2 == 1:
                            for e in range(2):
                                nc.tensor.matmul(
                                    pv[e][0:65, 0:NT],
                                    v8[:, i // 2, :, 0:65],
                                    p8[:, :, e, :],
                                    start=(i == 1),
                                    stop=False,
                                    perf_mode=mybir.MatmulPerfMode.DoubleRow,
                                )
                # normalize: o = pv[0:64] / pv[64]
                for e in range(2):
                    # copy psum out early to release the PV bank
                    pvs = small.tile([65, NT], F32, tag="pvs", name=f"pvs{e}")
                    nc.vector.tensor_copy(pvs[:], pv[e][0:65, :])
                    # reciprocal_approx_fast and partition_broadcast
                    # both require absolute partition 0 on HW: shift
                    # the denominator row down first.  The last block's
                    # normalize runs post-exp-stream: use the idle scalar
                    # ring there so the busy sync ring doesn't gate it.
                    deng = nc.scalar if tail else nc.sync
                    l0 = small.tile([1, NT], F32, tag="l0")
                    deng.dma_start(l0[:], pvs[64:65, :])
                    rec0 = small.tile([1, NT], F32, tag="rec0")
                    nc.vector.reciprocal_approx_fast(rec0[:], l0[:])
                    bca = small.tile([64, NT], F32, tag="bca")
                    nc.gpsimd.partition_broadcast(bca[:], rec0[:])
                    if e == 0:
                        nc.vector.tensor_tensor(
                            o_t[0:64, p_, j * NT : (j + 1) * NT],
                            pvs[0:64, :],
                            bca[:],
                            MULT,
                        )
                    else:
                        otmp = small.tile([64, NT], ADT, tag="otmp")
                        nc.vector.tensor_tensor(
                            otmp[:], pvs[0:64, :], bca[:], MULT
                        )
                        # o_t high-half shift only gates projection fillers
                        # (latency-tolerant): use the gpsimd SWDGE ring
                        # mid-stream to decongest the sync ring's outp/l0
                        # traffic; the last block stays on the idle scalar
                        # ring (tail latency matters there)
                        oeng = nc.scalar if tail else nc.gpsimd
                        oeng.dma_start(
                            o_t[64:128, p_, j * NT : (j + 1) * NT], otmp[:]
                        )

            def emit_v8(b, st8, n):
                # fp8 copy of V tile n (off the startup critical path; tile 3
                # never feeds a non-diagonal DoubleRow block, so n < 3 only)
                v_a, v8 = st8[4], st8[6]
                for i in range(4 * n, 4 * n + 4):
                    nc.vector.tensor_copy(
                        v8[:, i // 2, i % 2, 0:64], v_a[:, i, 0:64]
                    )

            def emit_proj_t(b, st8, t_, tail=False):
                # ---- output projection for one tq chunk (partial) ----
                # tail=True (post-exp-stream only): stage copies go to the
                # now-idle ACT engine so the DVE doesn't gate the drain
                o_t = st8[5]
                stage = stg.tile([P, C], F16, tag="stage")
                for n2 in range(2):
                    pm = ps_mm.tile([P, NT], F32, tag="mm")
                    for cc2 in range(2):
                        nc.tensor.matmul(
                            pm[:],
                            o_t[:, cc2, t_ * P : (t_ + 1) * P],
                            wproj_t[:, cc2, n2 * NT : (n2 + 1) * NT],
                            start=(cc2 == 0),
                            stop=(cc2 == 1),
                        )
                    if tail and n2 == 0:
                        nc.scalar.copy(stage[:, n2 * NT : (n2 + 1) * NT], pm[:])
                    else:
                        nc.vector.tensor_copy(
                            stage[:, n2 * NT : (n2 + 1) * NT], pm[:]
                        )
                nc.sync.dma_start(outp[b, t_ * P : (t_ + 1) * P, :], stage[:])

            # Emission order is BOTH program order (dependencies resolve by
            # it) and scheduler priority.  QKV tiles / next-batch setup must
            # therefore be emitted before the attention blocks that consume
            # them; projection chunks are pure consumers of o_t and can be
            # deferred — batch 0's late projections are emitted inside batch
            # 1's attention so they soak up PE idle slots there instead of
            # crowding the batch seam.
            def schedule(b, st8, nxt, prev_proj):
                QK = lambda n, p: ("qkv", n, p)
                PR = lambda t: ("prproj", t)
                MY = lambda t: ("myproj", t)
                SU = ("setup",)
                NX = lambda n, p: ("qkvn", n, p)
                V8 = lambda n: ("v8", n)
                if nxt is not None:
                    plan = {
                        (0, 0): [QK(1, 0), V8(0)],
                        (0, 1): [QK(1, 1), QK(1, 2)],
                        (1, 0): [QK(2, 0), QK(2, 1), V8(1)],
                        (1, 1): [QK(2, 2), QK(3, 0)],
                        (2, 0): [QK(3, 1), QK(3, 2), V8(2)],
                        (2, 1): [SU],
                        (3, 0): [NX(0, 0), NX(0, 1)],
                        (3, 1): [NX(0, 2), NX(1, 0), NX(1, 1)],
                    }
                else:
                    plan = {
                        (0, 0): [QK(1, 2), V8(0)],
                        (0, 1): [PR(0), PR(1)],
                        (1, 0): [QK(2, 0), PR(2), PR(3), V8(1)],
                        (1, 1): [QK(2, 1), QK(2, 2), PR(4), PR(5)],
                        (2, 0): [QK(3, 0), QK(3, 1), PR(6), PR(7), V8(2)],
                        (2, 1): [QK(3, 2), PR(8), PR(9), MY(0), MY(1)],
                        (3, 0): [PR(10), PR(11), MY(2), MY(3)],
                        (3, 1): [PR(12), PR(13), PR(14), PR(15), MY(4),
                                 MY(5), MY(6), MY(7), MY(8), MY(9), MY(10),
                                 MY(11)],
                    }
                pb, pst = (None, None) if prev_proj is None else prev_proj
                st_n = None
                for j in range(TQT):
                    for p_ in range(NPAIR):
                        emit_attn_jp(
                            b, st8, j, p_,
                            tail=(nxt is None and j == 3 and p_ == 1),
                        )
                        for f in plan[(j, p_)]:
                            if f[0] == "qkv":
                                emit_qkv_part(b, st8, f[1], f[2])
                            elif f[0] == "prproj":
                                emit_proj_t(pb, pst, f[1])
                            elif f[0] == "v8":
                                emit_v8(b, st8, f[1])
                            elif f[0] == "myproj":
                                emit_proj_t(b, st8, f[1])
                            elif f[0] == "setup" and nxt is not None:
                                st_n = emit_setup(nxt)
                            elif f[0] == "qkvn" and nxt is not None:
                                emit_qkv_part(nxt, st_n, f[1], f[2])
                if nxt is None:
                    for t_ in range(12, 16):
                        emit_proj_t(b, st8, t_, tail=True)
                return st_n

            st0 = emit_setup(0, first=True)
            emit_wproj_load()
            emit_qkv_part(0, st0, 0, 0)
            emit_qkv_part(0, st0, 0, 1)
            emit_qkv_part(0, st0, 0, 2)
            st1 = schedule(0, st0, 1, None)
            schedule(1, st1, None, (0, st0))

    nc.compile()
    return nc


_NC = None


def _get_program():
    global _NC
    if _NC is None:
        _NC = _build_program()
    return _NC


def _host_inputs(x, Wq, Wkv, Wproj):
    """Shard + lay out inputs for the 8 cores."""
    import ml_dtypes

    adt_np = ml_dtypes.bfloat16
    ident2 = np.concatenate([np.eye(64, dtype=np.float32)] * 2, axis=0).astype(
        adt_np
    )  # [128, 64]
    # additive causal mask (stationary operand): maska[q, k] = NEG iff k > q
    maska = np.where(
        np.arange(P)[None, :] > np.arange(P)[:, None], NEG, 0.0
    ).astype(adt_np)
    i128 = np.eye(P, dtype=np.float32).astype(adt_np)

    in_maps = []
    for d in range(2):
        # [2, P, CC, T]: xT[b, p, cc, t] = x[b, t, cc*128+p]
        xT = np.ascontiguousarray(
            x[2 * d : 2 * d + 2]
            .transpose(0, 2, 1)
            .reshape(2, CC, P, T)
            .transpose(0, 2, 1, 3)
        ).astype(adt_np)
        for g in range(G):
            heads = [g, g + 4, g + 8, g + 12]
            wq_cols = np.concatenate(
                [Wq[h * D : (h + 1) * D, :] for h in heads], axis=0
            ).T  # [1024, 256]
            wk = Wkv[g * D : (g + 1) * D, :].T  # [1024, 64]
            wv = Wkv[G * D + g * D : G * D + (g + 1) * D, :].T
            wqkv = np.ascontiguousarray(
                np.concatenate([wq_cols, wk, wv], axis=1)
                .reshape(CC, P, 384)
                .transpose(1, 0, 2)
            ).astype(adt_np)
            ch = np.concatenate(
                [np.arange(h * D, (h + 1) * D) for h in heads]
            )
            wproj_s = np.ascontiguousarray(
                Wproj[:, ch].T.reshape(2, P, C).transpose(1, 0, 2)
            ).astype(adt_np)
            in_maps.append(
                {
                    "xT": xT,
                    "wqkv": wqkv,
                    "wproj": wproj_s,
                    "maska": maska,
                    "i128": i128,
                    "ident2": ident2,
                    "vones": np.ones((P, TKC), dtype=adt_np),
                    "vones8": np.ones(
                        (P, TKC // 2, 2), dtype=ml_dtypes.float8_e4m3fn
                    ),
                }
            )
    return in_maps


def kernel(x, Wq, Wkv, Wproj, b_proj):
    x = np.asarray(x, dtype=np.float32)
    Wq = np.asarray(Wq, dtype=np.float32)
    Wkv = np.asarray(Wkv, dtype=np.float32)
    Wproj = np.asarray(Wproj, dtype=np.float32)
    b_proj = np.asarray(b_proj, dtype=np.float32)

    nc = _get_program()
    in_maps = _host_inputs(x, Wq, Wkv, Wproj)
    trace = bool(int(os.environ.get("BASS_KERNEL_TRACE", "0")))
    res = run_bass_kernel_spmd(nc, in_maps, list(range(8)), trace=trace)
    if trace:
        kernel.last_results = res

    out = np.empty((B, T, C), dtype=np.float32)
    for d in range(2):
        acc = res.results[4 * d]["outp"].astype(np.float32)
        for g in range(1, G):
            acc += res.results[4 * d + g]["outp"].astype(np.float32)
        out[2 * d : 2 * d + 2] = acc + b_proj[None, None, :]
    return out
